# revision 1
# baseline (speedup 1.0000x reference)
"""Trainium2 Bass kernel for nn_MultiHeadAttention_64037962383811.

Reference (per batch b):
  q = x @ Wq[h].T + bq[h];  k = states @ Wk[h].T + bk[h];  v = states @ Wv[h].T + bv[h]
  scores = q k^T / sqrt(512);  masked softmax over Lk;  ctx = attn @ v
  out = concat_h(ctx) @ Wp.T + bp

Sharding: data-parallel over batch B=8 -> one batch element per NeuronCore
(8 cores). No collectives; each core computes its full [1024, 512] output
slice and the host stacks them.

Per-core dataflow (projection/score/output matmuls in float32r = TF32-class
at full PE rate; P^T and V in fp16 -> same PE rate, ~3e-4 rounding):
  - Everything is kept in "transposed" layouts so that the PE's
    partition-dim contraction lines up with zero on-chip transposes:
      x^T, states^T        [e, l]   (host-transposed)
      Q^T, K^T = W^T @ x^T [d, l]   (from projection matmuls directly)
      S^T = K Q^T          [k, q]   (scores, transposed)
      P^T = exp(S^T) * m^T          (mask host-transposed, fp16)
      rowsum = ones^T @ (sum_kj P^T) [1, q]  (DVE-accumulated, one
                                     partition-reduce matmul per q-block)
      ctx^T = V^T P^T      [d, q]   (V kept [k, d], natural)
      out = ctx_n^T.T @ Wp^T [q, o] (accumulated over heads in SBUF)
  - Softmax without max-subtraction (scores ~ N(0,1), exp is safe) and
    without -inf masking: P = exp(S) * mask, normalized by rowsum(P).
  - Division via reciprocal_approx_fast + gpsimd partition-broadcast.

The head loop is software-pipelined: iteration h emits [proj(h),
outproj(h-1), attn(h)] so the PE never waits on the softmax/normalize
tail of the previous head. Bias matmuls are compiled out when all bias
vectors are zero (they are, for this problem's setup_inputs).
"""
import sys

for _p in (
    "/root/.axon_site",
    "/root/.axon_site/_ro/trn_rl_repo",
    "/root/.axon_site/_ro/pypackages",
):
    if _p not in sys.path:
        sys.path.insert(0, _p)

import numpy as np
import ml_dtypes
from contextlib import ExitStack

import concourse.bacc as bacc
import concourse.tile as tile
import concourse.mybir as mybir
from concourse.bass_utils import run_bass_kernel_spmd

B, L, E, D, H = 8, 1024, 512, 512, 8
NCORES = 8
F32 = mybir.dt.float32
F32R = mybir.dt.float32r
F16 = mybir.dt.float16
AF = mybir.ActivationFunctionType
SCALE = float(1.0 / np.sqrt(E))

PT_BUFS = 9  # P^T sbuf tiles in flight (8 needed live per (h, qb))

TRACE = False  # test harness sets kernel.TRACE = True to profile
LAST_EXEC_NS = None

_cache = {}


def _build(use_bias):
    nc = bacc.Bacc("TRN2", target_bir_lowering=False, debug=False)

    xT_d = nc.dram_tensor("xT", [4, 128, L], F16, kind="ExternalInput").ap()
    sT_d = nc.dram_tensor("sT", [4, 128, L], F16, kind="ExternalInput").ap()
    mk_d = nc.dram_tensor("maskT", [8, 128, L], F16, kind="ExternalInput").ap()
    wq_d = nc.dram_tensor("wqT", [H, 4, 128, D], F16, kind="ExternalInput").ap()
    wk_d = nc.dram_tensor("wkT", [H, 4, 128, D], F16, kind="ExternalInput").ap()
    wv_d = nc.dram_tensor("wvT", [H, 4, 128, D], F16, kind="ExternalInput").ap()
    wp_d = nc.dram_tensor("wpT", [H, 4, 128, D], F16, kind="ExternalInput").ap()
    if use_bias:
        bq_d = nc.dram_tensor("bq", [H, D], F32R, kind="ExternalInput").ap()
        bk_d = nc.dram_tensor("bk", [H, D], F32R, kind="ExternalInput").ap()
        bv_d = nc.dram_tensor("bv", [H, D], F32R, kind="ExternalInput").ap()
        bp_d = nc.dram_tensor("bp", [1, D], F32R, kind="ExternalInput").ap()
    on_d = nc.dram_tensor("ones", [128, 512], F32R, kind="ExternalInput").ap()
    out_d = nc.dram_tensor("out", [L, D], F32, kind="ExternalOutput").ap()

    with tile.TileContext(nc) as tc, ExitStack() as ctx:
        const = ctx.enter_context(tc.tile_pool(name="const", bufs=1))
        wpool = ctx.enter_context(tc.tile_pool(name="w", bufs=1))
        qkv = ctx.enter_context(tc.tile_pool(name="qkv", bufs=1))
        ptp = ctx.enter_context(tc.tile_pool(name="ptp", bufs=PT_BUFS))
        ctxp = ctx.enter_context(tc.tile_pool(name="ctxp", bufs=1))
        small = ctx.enter_context(tc.tile_pool(name="small", bufs=2))
        psum = ctx.enter_context(tc.tile_pool(name="ps", bufs=7, space="PSUM"))
        psrow = ctx.enter_context(tc.tile_pool(name="psrow", bufs=1, space="PSUM"))

        # Resident tiles. DMA emission order matters for startup latency:
        # head-0 weights + xT/sT go first, bulky mask/bias loads after the
        # first projections are emitted.
        mask_sb = const.tile([128, 8, L], F16, tag="mask")
        xT = const.tile([128, 4, L], F16, tag="xT")
        sT = const.tile([128, 4, L], F16, tag="sT")
        ones = const.tile([128, 512], F32R, tag="ones")
        out_acc = const.tile([128, 8, D], F32, tag="oacc")
        if use_bias:
            bp_sb = const.tile([1, D], F32R, tag="bp")

        # Tile orders dependencies by emission order: every read must be
        # emitted AFTER its writer, or it races with the DMA. proj(0)'s bias
        # matmuls read `ones`, so in the bias variant it must load up front.
        if use_bias:
            nc.sync.dma_start(ones[:], on_d)

        def load_consts():
            """Emitted after proj(0): non-critical-path resident loads.
            Everything here is first read in attn(0)/outproj(0) or later,
            which are emitted after this point."""
            nc.sync.dma_start(mask_sb[:], mk_d.transpose([1, 0, 2]))
            if not use_bias:
                nc.sync.dma_start(ones[:], on_d)
            if use_bias:
                nc.sync.dma_start(bp_sb[:], bp_d)

        state = {}

        def proj(h):
            """Q^T, K^T [128,4dj,L] and V [128,8kj,D] projections for head h."""
            wq = wpool.tile([128, 4, D], F16, tag="wq")
            wk = wpool.tile([128, 4, D], F16, tag="wk")
            wv = wpool.tile([128, 4, D], F16, tag="wv")
            if h == 0:
                # Fine-grained first loads: the first projection matmul only
                # needs (wq, xT) slab ej=0, so don't gate it on 3 MB of DMA.
                for ej in range(4):
                    nc.sync.dma_start(wq[:, ej, :], wq_d[h, ej])
                    nc.sync.dma_start(xT[:, ej, :], xT_d[ej])
                for ej in range(4):
                    nc.sync.dma_start(wk[:, ej, :], wk_d[h, ej])
                    nc.sync.dma_start(sT[:, ej, :], sT_d[ej])
            else:
                nc.sync.dma_start(wq[:], wq_d[h].transpose([1, 0, 2]))
                nc.sync.dma_start(wk[:], wk_d[h].transpose([1, 0, 2]))
            nc.sync.dma_start(wv[:], wv_d[h].transpose([1, 0, 2]))
            if use_bias:
                # One [1,128] tile per d-chunk: f32r stationary operands are
                # read at offset 0 of their tile (sliced lhsT offsets misload).
                bq_ts, bk_ts = [], []
                for j in range(4):
                    t = small.tile([1, 128], F32R, tag=f"bq{j}")
                    nc.sync.dma_start(t[:], bq_d[h : h + 1, j * 128 : (j + 1) * 128])
                    bq_ts.append(t)
                    t = small.tile([1, 128], F32R, tag=f"bk{j}")
                    nc.sync.dma_start(t[:], bk_d[h : h + 1, j * 128 : (j + 1) * 128])
                    bk_ts.append(t)
                bv_t = small.tile([1, D], F32R, tag="bv")
                nc.sync.dma_start(bv_t[:], bv_d[h : h + 1, :])

            qt = qkv.tile([128, 4, L], F16, tag="qt")
            kt = qkv.tile([128, 4, L], F16, tag="kt")
            vt = qkv.tile([128, 8, D], F16, tag="vt")
            # Q^T / K^T: out[d_tile, q] = sum_e W^T[e, d].T @ xT[e, q]
            # Bias (rare path): rank-1 matmul bias_slice^T @ ones_row adds
            # b[d] across the whole q free dim.
            for wmat, src, dst, which in (
                (wq, xT, qt, "q"),
                (wk, sT, kt, "k"),
            ):
                for qb in range(2):
                    for dj in range(4):
                        ps = psum.tile([128, 512], F32, tag="mm")
                        for ej in range(4):
                            nc.tensor.matmul(
                                ps[:],
                                wmat[:, ej, dj * 128 : (dj + 1) * 128],
                                src[:, ej, qb * 512 : (qb + 1) * 512],
                                start=(ej == 0),
                                stop=(ej == 3 and not use_bias),
                            )
                        if use_bias:
                            b_t = (bq_ts if which == "q" else bk_ts)[dj]
                            nc.tensor.matmul(
                                ps[:],
                                b_t[:],
                                ones[0:1, :],
                                start=False,
                                stop=True,
                            )
                        dsl = dst[:, dj, qb * 512 : (qb + 1) * 512]
                        nc.scalar.copy(dsl, ps[:])
            # V (bf16): out[k_tile, d] = sum_e sT[e, k].T @ Wv^T[e, d] (+ bv)
            for kj in range(8):
                ps = psum.tile([128, 512], F32, tag="mm")
                for ej in range(4):
                    nc.tensor.matmul(
                        ps[:],
                        sT[:, ej, kj * 128 : (kj + 1) * 128],
                        wv[:, ej, :],
                        start=(ej == 0),
                        stop=(ej == 3 and not use_bias),
                    )
                if use_bias:
                    nc.tensor.matmul(
                        ps[:], ones[0:1, 0:128], bv_t[:], start=False, stop=True
                    )
                nc.scalar.copy(vt[:, kj, :], ps[:])
            state[h] = {"qt": qt, "kt": kt, "vt": vt}

        def attn(h):
            """S^T -> exp*mask -> rowsum -> ctx^T -> normalize, per q-block."""
            st = state[h]
            qt, kt, vt = st["qt"], st["kt"], st["vt"]
            ctxn = ctxp.tile([128, 4, L], F16, tag="ctxn")
            for qb in range(2):
                qsl = slice(qb * 512, (qb + 1) * 512)
                pts = []
                acc = small.tile([128, 512], F32R, tag="acc")
                for kj in range(8):
                    ps = psum.tile([128, 512], F32, tag="mm")
                    for dc in range(4):
                        nc.tensor.matmul(
                            ps[:],
                            kt[:, dc, kj * 128 : (kj + 1) * 128],
                            qt[:, dc, qsl],
                            start=(dc == 0),
                            stop=(dc == 3),
                        )
                    pt = ptp.tile([128, 512], F16, tag="pt")
                    nc.scalar.activation(pt[:], ps[:], AF.Exp, scale=SCALE)
                    nc.vector.tensor_mul(pt[:], pt[:], mask_sb[:, kj, qsl])
                    if kj == 0:
                        nc.vector.tensor_copy(acc[:], pt[:])
                    else:
                        nc.vector.tensor_add(acc[:], acc[:], pt[:])
                    pts.append(pt)
                rs = psrow.tile([1, 512], F32, tag="row")
                nc.tensor.matmul(
                    rs[:], ones[:, 0:1], acc[:], start=True, stop=True
                )
                rec = small.tile([1, 512], F32, tag="rec")
                nc.vector.reciprocal_approx_fast(rec[:], rs[:])
                rb = small.tile([128, 512], F32, tag="rb")
                nc.gpsimd.partition_broadcast(rb[:], rec[:])
                for dj in range(4):
                    cps = psum.tile([128, 512], F32, tag="mm")
                    for kj in range(8):
                        nc.tensor.matmul(
                            cps[:],
                            vt[:, kj, dj * 128 : (dj + 1) * 128],
                            pts[kj][:],
                            start=(kj == 0),
                            stop=(kj == 7),
                        )
                    nc.vector.tensor_mul(ctxn[:, dj, qsl], cps[:], rb[:])
            state[h]["ctxn"] = ctxn

        def outproj(h):
            """out_acc[q, o] += sum_dj ctx_n^T[i, q].T @ Wp^T[i, o]."""
            wp = wpool.tile([128, 4, D], F16, tag="wp")
            for dj in range(4):
                nc.sync.dma_start(wp[:, dj, :], wp_d[h, dj])
            ctxn = state[h]["ctxn"]
            for qm in range(8):
                ps = psum.tile([128, 512], F32, tag="mm")
                for dj in range(4):
                    nc.tensor.matmul(
                        ps[:],
                        ctxn[:, dj, qm * 128 : (qm + 1) * 128],
                        wp[:, dj, :],
                        start=(dj == 0),
                        stop=(dj == 3 and not (h == 0 and use_bias)),
                    )
                if h == 0 and use_bias:
                    nc.tensor.matmul(
                        ps[:], ones[0:1, 0:128], bp_sb[:], start=False, stop=True
                    )
                if h == 0:
                    nc.scalar.copy(out_acc[:, qm, :], ps[:])
                else:
                    nc.vector.tensor_add(
                        out_acc[:, qm, :], out_acc[:, qm, :], ps[:]
                    )
                if h == H - 1:
                    nc.sync.dma_start(
                        out_d[qm * 128 : (qm + 1) * 128, :], out_acc[:, qm, :]
                    )
            del state[h]["qt"], state[h]["kt"], state[h]["vt"], state[h]["ctxn"]

        for h in range(H):
            proj(h)
            if h == 0:
                load_consts()
            if h > 0:
                outproj(h - 1)
            attn(h)
        outproj(H - 1)

    nc.compile()
    return nc


def _get_program(use_bias):
    key = ("nc", use_bias)
    if key not in _cache:
        _cache[key] = _build(use_bias)
    return _cache[key]


def kernel(x, states, mask, Wq, bq, Wk, bk, Wv, bv, Wp, bp):
    global LAST_EXEC_NS

    x = np.asarray(x, dtype=np.float32)
    states = np.asarray(states, dtype=np.float32)
    mask = np.asarray(mask)
    f32 = np.float32
    bq_np, bk_np = np.asarray(bq, f32), np.asarray(bk, f32)
    bv_np, bp_np = np.asarray(bv, f32), np.asarray(bp, f32)
    use_bias = bool(
        bq_np.any() or bk_np.any() or bv_np.any() or bp_np.any()
    )
    nc = _get_program(use_bias)

    wq_np = np.ascontiguousarray(
        np.asarray(Wq, f32).transpose(0, 2, 1)
    ).reshape(H, 4, 128, D).astype(np.float16)
    wk_np = np.ascontiguousarray(
        np.asarray(Wk, f32).transpose(0, 2, 1)
    ).reshape(H, 4, 128, D).astype(np.float16)
    wv_np = np.ascontiguousarray(
        np.asarray(Wv, f32).transpose(0, 2, 1)
    ).reshape(H, 4, 128, D).astype(np.float16)
    wp_np = np.ascontiguousarray(np.asarray(Wp, f32).T).reshape(H, 4, 128, D).astype(np.float16)

    shared = {
        "wqT": wq_np,
        "wkT": wk_np,
        "wvT": wv_np,
        "wpT": wp_np,
        "ones": np.ones((128, 512), f32),
    }
    if use_bias:
        shared["bq"] = bq_np
        shared["bk"] = bk_np
        shared["bv"] = bv_np
        shared["bp"] = bp_np.reshape(1, D)

    in_maps = []
    for b in range(B):
        xT = np.ascontiguousarray(x[b].T).reshape(4, 128, L).astype(np.float16)
        sT = np.ascontiguousarray(states[b].T).reshape(4, 128, L).astype(np.float16)
        mT = np.ascontiguousarray(mask[b].T).astype(np.float16).reshape(
            8, 128, L
        )
        in_maps.append({"xT": xT, "sT": sT, "maskT": mT, **shared})

    res = run_bass_kernel_spmd(
        nc, in_maps, core_ids=list(range(NCORES)), trace=TRACE
    )
    LAST_EXEC_NS = res.exec_time_ns
    return np.stack([res.results[b]["out"] for b in range(B)], axis=0)



# revision 2
# speedup vs baseline: 1.1277x; 1.1277x over previous
"""Trainium2 Bass kernel for nn_MultiHeadAttention_64037962383811.

Reference (per batch b):
  q = x @ Wq[h].T;  k = states @ Wk[h].T;  v = states @ Wv[h].T
  scores = q k^T / sqrt(512);  masked softmax over Lk;  ctx = attn @ v
  out = concat_h(ctx) @ Wp.T + bp

Weight-folding trick (zero-bias fast path): the Q/K and V/out projections
collapse into per-head combined matrices
  M_h = Wq[h].T @ Wk[h]   [e,e']   (S = x M_h states^T / sqrt(512))
  N_h = Wv[h].T @ Wp_h.T  [e,o]    (out += (P @ states) @ N_h)
so per head only ONE x-side projection (T = x M_h) and ONE output-side
projection ((P states) N_h) remain, plus the two L x L attention matmuls.
Per-core MACs drop from 17.2G (QKV+attn+out) to 15.4G (incl. the 2.1G
redundant M/N computation).

Sharding: data-parallel over batch B=8 -> one batch element per NeuronCore
(8 cores). No collectives; each core computes its full [1024, 512] output
slice and the host stacks them.

Per-core dataflow, all in transposed layouts (zero on-chip transposes):
  x^T, states^T     [e, l]  (host-transposed)
  states_nat        [k, e]  (host natural-chunked; replaces V)
  M_h = Wq.T Wk     [e, e'] (from PE directly: lhsT=Wq[d,e], rhs=Wk[d,e'])
  N_h = Wv.T Wp_h.T [e, o]  (lhsT=Wv[d,e], rhs=Wp_h.T[d,o])
  T^T = M^T x^T     [e', q]
  S^T = states T^T  [k, q]
  P^T = exp(S^T) * m^T      (mask host-transposed, fp16)
  rowsum = ones^T @ (sum_kj P^T) [1, q]  (DVE-accumulated partition-reduce)
  G^T = states_nat^T P^T    [e, q]  (= (P @ states)^T)
  out[q,o] += G^T.T @ N_h   (accumulated over heads in SBUF)
Softmax without max-subtraction (scores ~ N(0,1)) and without -inf
masking: P = exp(S) * mask, normalized by rowsum(P) applied to G^T.

The head loop is software-pipelined: iteration h emits [MN+T(h),
outproj(h-1), attn(h)] so the PE never waits on the softmax tail of the
previous head. A nonzero-bias fallback runs the original unfolded kernel.
"""
import sys

for _p in (
    "/root/.axon_site",
    "/root/.axon_site/_ro/trn_rl_repo",
    "/root/.axon_site/_ro/pypackages",
):
    if _p not in sys.path:
        sys.path.insert(0, _p)

import numpy as np
import ml_dtypes
from contextlib import ExitStack

import concourse.bacc as bacc
import concourse.tile as tile
import concourse.mybir as mybir
from concourse.bass_utils import run_bass_kernel_spmd

B, L, E, D, H = 8, 1024, 512, 512, 8
NCORES = 8
F32 = mybir.dt.float32
F32R = mybir.dt.float32r
F16 = mybir.dt.float16
AF = mybir.ActivationFunctionType
SCALE = float(1.0 / np.sqrt(E))

PT_BUFS = 9  # P^T sbuf tiles in flight (8 needed live per (h, qb))

TRACE = False  # test harness sets kernel.TRACE = True to profile
LAST_EXEC_NS = None

_cache = {}


def _build_fast():
    """Zero-bias fast path with per-head folded weights M_h, N_h."""
    nc = bacc.Bacc("TRN2", target_bir_lowering=False, debug=False)

    xT_d = nc.dram_tensor("xT", [4, 128, L], F16, kind="ExternalInput").ap()
    sT_d = nc.dram_tensor("sT", [4, 128, L], F16, kind="ExternalInput").ap()
    sN_d = nc.dram_tensor("sN", [8, 128, E], F16, kind="ExternalInput").ap()
    mk_d = nc.dram_tensor("maskT", [8, 128, L], F16, kind="ExternalInput").ap()
    wq_d = nc.dram_tensor("wq", [H, 4, 128, E], F16, kind="ExternalInput").ap()
    wk_d = nc.dram_tensor("wk", [H, 4, 128, E], F16, kind="ExternalInput").ap()
    wv_d = nc.dram_tensor("wv", [H, 4, 128, E], F16, kind="ExternalInput").ap()
    wp_d = nc.dram_tensor("wpT", [H, 4, 128, D], F16, kind="ExternalInput").ap()
    on_d = nc.dram_tensor("ones", [128, 512], F32R, kind="ExternalInput").ap()
    out_d = nc.dram_tensor("out", [L, D], F32, kind="ExternalOutput").ap()

    with tile.TileContext(nc) as tc, ExitStack() as ctx:
        const = ctx.enter_context(tc.tile_pool(name="const", bufs=1))
        wpool = ctx.enter_context(tc.tile_pool(name="w", bufs=2))
        mnp = ctx.enter_context(tc.tile_pool(name="mn", bufs=2))
        ttp = ctx.enter_context(tc.tile_pool(name="tt", bufs=2))
        ptp = ctx.enter_context(tc.tile_pool(name="ptp", bufs=PT_BUFS))
        ctxp = ctx.enter_context(tc.tile_pool(name="ctxp", bufs=2))
        small = ctx.enter_context(tc.tile_pool(name="small", bufs=2))
        psum = ctx.enter_context(tc.tile_pool(name="ps", bufs=7, space="PSUM"))
        psrow = ctx.enter_context(tc.tile_pool(name="psrow", bufs=1, space="PSUM"))

        # Resident tiles. DMA emission order matters for startup latency:
        # head-0 weights go first, bulky mask/states loads after the first
        # M/N chains are emitted.
        mask_sb = const.tile([128, 8, L], F16, tag="mask")
        xT = const.tile([128, 4, L], F16, tag="xT")
        sT = const.tile([128, 4, L], F16, tag="sT")
        sN = const.tile([128, 8, E], F16, tag="sN")
        ones = const.tile([128, 512], F32R, tag="ones")
        out_acc = const.tile([128, 8, D], F32, tag="oacc")

        def load_consts():
            """Emitted after MN(0)+T(0): non-critical-path resident loads.
            Everything here is first read in attn(0) or later, which are
            emitted after this point. Order = first-use order."""
            nc.sync.dma_start(sT[:], sT_d.transpose([1, 0, 2]))
            nc.sync.dma_start(mask_sb[:], mk_d.transpose([1, 0, 2]))
            nc.sync.dma_start(sN[:], sN_d.transpose([1, 0, 2]))
            nc.sync.dma_start(ones[:], on_d)

        state = {}

        def mn_t(h):
            """Folded weights M_h [e,4ej',E], N_h [e,4ej,o] and T^T [e',4,L]."""
            wq = wpool.tile([128, 4, E], F16, tag="wq")
            wk = wpool.tile([128, 4, E], F16, tag="wk")
            wv = wpool.tile([128, 4, E], F16, tag="wv")
            wp = wpool.tile([128, 4, D], F16, tag="wp")
            if h == 0:
                # Fine-grained first loads: the first M chain needs all wq/wk
                # d-chunks; interleave so both fill together, then x^T.
                for dj in range(4):
                    nc.sync.dma_start(wq[:, dj, :], wq_d[h, dj])
                    nc.sync.dma_start(wk[:, dj, :], wk_d[h, dj])
                for dj in range(4):
                    nc.sync.dma_start(wv[:, dj, :], wv_d[h, dj])
                    nc.sync.dma_start(wp[:, dj, :], wp_d[h, dj])
                for ej in range(4):
                    nc.sync.dma_start(xT[:, ej, :], xT_d[ej])
            else:
                nc.sync.dma_start(wq[:], wq_d[h].transpose([1, 0, 2]))
                nc.sync.dma_start(wk[:], wk_d[h].transpose([1, 0, 2]))
                nc.sync.dma_start(wv[:], wv_d[h].transpose([1, 0, 2]))
                nc.sync.dma_start(wp[:], wp_d[h].transpose([1, 0, 2]))

            mt = mnp.tile([128, 4, E], F16, tag="mt")
            nt = mnp.tile([128, 4, D], F16, tag="nt")
            # M[e-chunk ej', e'] = sum_d Wq[d, e-chunk].T @ Wk[d, e']
            for ej in range(4):
                ps = psum.tile([128, 512], F32, tag="mm")
                for dj in range(4):
                    nc.tensor.matmul(
                        ps[:],
                        wq[:, dj, ej * 128 : (ej + 1) * 128],
                        wk[:, dj, :],
                        start=(dj == 0),
                        stop=(dj == 3),
                    )
                nc.scalar.copy(mt[:, ej, :], ps[:])
            # N[e-chunk ej, o] = sum_d Wv[d, e-chunk].T @ Wp_h.T[d, o]
            for ej in range(4):
                ps = psum.tile([128, 512], F32, tag="mm")
                for dj in range(4):
                    nc.tensor.matmul(
                        ps[:],
                        wv[:, dj, ej * 128 : (ej + 1) * 128],
                        wp[:, dj, :],
                        start=(dj == 0),
                        stop=(dj == 3),
                    )
                nc.scalar.copy(nt[:, ej, :], ps[:])
            # T^T[e'-chunk ej', q] = sum_e M[e, e'-chunk].T @ x^T[e, q]
            tt = ttp.tile([128, 4, L], F16, tag="tt")
            for qb in range(2):
                for ej2 in range(4):
                    ps = psum.tile([128, 512], F32, tag="mm")
                    for ej in range(4):
                        nc.tensor.matmul(
                            ps[:],
                            mt[:, ej, ej2 * 128 : (ej2 + 1) * 128],
                            xT[:, ej, qb * 512 : (qb + 1) * 512],
                            start=(ej == 0),
                            stop=(ej == 3),
                        )
                    nc.scalar.copy(tt[:, ej2, qb * 512 : (qb + 1) * 512], ps[:])
            state[h] = {"tt": tt, "nt": nt}

        def attn(h):
            """S^T -> exp*mask -> rowsum -> G^T -> normalize, per q-block."""
            st = state[h]
            tt = st["tt"]
            gt = ctxp.tile([128, 4, L], F16, tag="gt")
            for qb in range(2):
                qsl = slice(qb * 512, (qb + 1) * 512)
                pts = []
                acc = small.tile([128, 512], F32R, tag="acc")
                for kj in range(8):
                    ps = psum.tile([128, 512], F32, tag="mm")
                    for dc in range(4):
                        nc.tensor.matmul(
                            ps[:],
                            sT[:, dc, kj * 128 : (kj + 1) * 128],
                            tt[:, dc, qsl],
                            start=(dc == 0),
                            stop=(dc == 3),
                        )
                    pt = ptp.tile([128, 512], F16, tag="pt")
                    nc.scalar.activation(pt[:], ps[:], AF.Exp, scale=SCALE)
                    nc.vector.tensor_mul(pt[:], pt[:], mask_sb[:, kj, qsl])
                    if kj == 0:
                        nc.vector.tensor_copy(acc[:], pt[:])
                    else:
                        nc.vector.tensor_add(acc[:], acc[:], pt[:])
                    pts.append(pt)
                rs = psrow.tile([1, 512], F32, tag="row")
                nc.tensor.matmul(
                    rs[:], ones[:, 0:1], acc[:], start=True, stop=True
                )
                rec = small.tile([1, 512], F32, tag="rec")
                nc.vector.reciprocal_approx_fast(rec[:], rs[:])
                rb = small.tile([128, 512], F32, tag="rb")
                nc.gpsimd.partition_broadcast(rb[:], rec[:])
                # G^T[e-chunk dj, q] = sum_k states_nat[k, e-chunk].T @ P^T
                for dj in range(4):
                    cps = psum.tile([128, 512], F32, tag="mm")
                    for kj in range(8):
                        nc.tensor.matmul(
                            cps[:],
                            sN[:, kj, dj * 128 : (dj + 1) * 128],
                            pts[kj][:],
                            start=(kj == 0),
                            stop=(kj == 7),
                        )
                    nc.vector.tensor_mul(gt[:, dj, qsl], cps[:], rb[:])
            state[h]["gt"] = gt

        def outproj(h):
            """out_acc[q, o] += sum_dj G^T[e, q].T @ N_h[e, o]."""
            st = state[h]
            gt, nt = st["gt"], st["nt"]
            for qm in range(8):
                ps = psum.tile([128, 512], F32, tag="mm")
                for dj in range(4):
                    nc.tensor.matmul(
                        ps[:],
                        gt[:, dj, qm * 128 : (qm + 1) * 128],
                        nt[:, dj, :],
                        start=(dj == 0),
                        stop=(dj == 3),
                    )
                if h == 0:
                    nc.scalar.copy(out_acc[:, qm, :], ps[:])
                else:
                    nc.vector.tensor_add(
                        out_acc[:, qm, :], out_acc[:, qm, :], ps[:]
                    )
                if h == H - 1:
                    nc.sync.dma_start(
                        out_d[qm * 128 : (qm + 1) * 128, :], out_acc[:, qm, :]
                    )
            del state[h]["tt"], state[h]["nt"], state[h]["gt"]

        for h in range(H):
            mn_t(h)
            if h == 0:
                load_consts()
            if h > 0:
                outproj(h - 1)
            attn(h)
        outproj(H - 1)

    nc.compile()
    return nc


def _build_bias():
    """Original unfolded kernel — fallback for nonzero biases."""
    use_bias = True
    nc = bacc.Bacc("TRN2", target_bir_lowering=False, debug=False)

    xT_d = nc.dram_tensor("xT", [4, 128, L], F16, kind="ExternalInput").ap()
    sT_d = nc.dram_tensor("sT", [4, 128, L], F16, kind="ExternalInput").ap()
    mk_d = nc.dram_tensor("maskT", [8, 128, L], F16, kind="ExternalInput").ap()
    wq_d = nc.dram_tensor("wqT", [H, 4, 128, D], F16, kind="ExternalInput").ap()
    wk_d = nc.dram_tensor("wkT", [H, 4, 128, D], F16, kind="ExternalInput").ap()
    wv_d = nc.dram_tensor("wvT", [H, 4, 128, D], F16, kind="ExternalInput").ap()
    wp_d = nc.dram_tensor("wpT", [H, 4, 128, D], F16, kind="ExternalInput").ap()
    bq_d = nc.dram_tensor("bq", [H, D], F32R, kind="ExternalInput").ap()
    bk_d = nc.dram_tensor("bk", [H, D], F32R, kind="ExternalInput").ap()
    bv_d = nc.dram_tensor("bv", [H, D], F32R, kind="ExternalInput").ap()
    bp_d = nc.dram_tensor("bp", [1, D], F32R, kind="ExternalInput").ap()
    on_d = nc.dram_tensor("ones", [128, 512], F32R, kind="ExternalInput").ap()
    out_d = nc.dram_tensor("out", [L, D], F32, kind="ExternalOutput").ap()

    with tile.TileContext(nc) as tc, ExitStack() as ctx:
        const = ctx.enter_context(tc.tile_pool(name="const", bufs=1))
        wpool = ctx.enter_context(tc.tile_pool(name="w", bufs=1))
        qkv = ctx.enter_context(tc.tile_pool(name="qkv", bufs=1))
        ptp = ctx.enter_context(tc.tile_pool(name="ptp", bufs=PT_BUFS))
        ctxp = ctx.enter_context(tc.tile_pool(name="ctxp", bufs=1))
        small = ctx.enter_context(tc.tile_pool(name="small", bufs=2))
        psum = ctx.enter_context(tc.tile_pool(name="ps", bufs=7, space="PSUM"))
        psrow = ctx.enter_context(tc.tile_pool(name="psrow", bufs=1, space="PSUM"))

        mask_sb = const.tile([128, 8, L], F16, tag="mask")
        xT = const.tile([128, 4, L], F16, tag="xT")
        sT = const.tile([128, 4, L], F16, tag="sT")
        ones = const.tile([128, 512], F32R, tag="ones")
        out_acc = const.tile([128, 8, D], F32, tag="oacc")
        bp_sb = const.tile([1, D], F32R, tag="bp")

        nc.sync.dma_start(ones[:], on_d)

        def load_consts():
            nc.sync.dma_start(mask_sb[:], mk_d.transpose([1, 0, 2]))
            nc.sync.dma_start(bp_sb[:], bp_d)

        state = {}

        def proj(h):
            wq = wpool.tile([128, 4, D], F16, tag="wq")
            wk = wpool.tile([128, 4, D], F16, tag="wk")
            wv = wpool.tile([128, 4, D], F16, tag="wv")
            if h == 0:
                for ej in range(4):
                    nc.sync.dma_start(wq[:, ej, :], wq_d[h, ej])
                    nc.sync.dma_start(xT[:, ej, :], xT_d[ej])
                for ej in range(4):
                    nc.sync.dma_start(wk[:, ej, :], wk_d[h, ej])
                    nc.sync.dma_start(sT[:, ej, :], sT_d[ej])
            else:
                nc.sync.dma_start(wq[:], wq_d[h].transpose([1, 0, 2]))
                nc.sync.dma_start(wk[:], wk_d[h].transpose([1, 0, 2]))
            nc.sync.dma_start(wv[:], wv_d[h].transpose([1, 0, 2]))
            bq_ts, bk_ts = [], []
            for j in range(4):
                t = small.tile([1, 128], F32R, tag=f"bq{j}")
                nc.sync.dma_start(t[:], bq_d[h : h + 1, j * 128 : (j + 1) * 128])
                bq_ts.append(t)
                t = small.tile([1, 128], F32R, tag=f"bk{j}")
                nc.sync.dma_start(t[:], bk_d[h : h + 1, j * 128 : (j + 1) * 128])
                bk_ts.append(t)
            bv_t = small.tile([1, D], F32R, tag="bv")
            nc.sync.dma_start(bv_t[:], bv_d[h : h + 1, :])

            qt = qkv.tile([128, 4, L], F16, tag="qt")
            kt = qkv.tile([128, 4, L], F16, tag="kt")
            vt = qkv.tile([128, 8, D], F16, tag="vt")
            for wmat, src, dst, which in (
                (wq, xT, qt, "q"),
                (wk, sT, kt, "k"),
            ):
                for qb in range(2):
                    for dj in range(4):
                        ps = psum.tile([128, 512], F32, tag="mm")
                        for ej in range(4):
                            nc.tensor.matmul(
                                ps[:],
                                wmat[:, ej, dj * 128 : (dj + 1) * 128],
                                src[:, ej, qb * 512 : (qb + 1) * 512],
                                start=(ej == 0),
                                stop=False,
                            )
                        b_t = (bq_ts if which == "q" else bk_ts)[dj]
                        nc.tensor.matmul(
                            ps[:], b_t[:], ones[0:1, :], start=False, stop=True
                        )
                        dsl = dst[:, dj, qb * 512 : (qb + 1) * 512]
                        nc.scalar.copy(dsl, ps[:])
            for kj in range(8):
                ps = psum.tile([128, 512], F32, tag="mm")
                for ej in range(4):
                    nc.tensor.matmul(
                        ps[:],
                        sT[:, ej, kj * 128 : (kj + 1) * 128],
                        wv[:, ej, :],
                        start=(ej == 0),
                        stop=False,
                    )
                nc.tensor.matmul(
                    ps[:], ones[0:1, 0:128], bv_t[:], start=False, stop=True
                )
                nc.scalar.copy(vt[:, kj, :], ps[:])
            state[h] = {"qt": qt, "kt": kt, "vt": vt}

        def attn(h):
            st = state[h]
            qt, kt, vt = st["qt"], st["kt"], st["vt"]
            ctxn = ctxp.tile([128, 4, L], F16, tag="ctxn")
            for qb in range(2):
                qsl = slice(qb * 512, (qb + 1) * 512)
                pts = []
                acc = small.tile([128, 512], F32R, tag="acc")
                for kj in range(8):
                    ps = psum.tile([128, 512], F32, tag="mm")
                    for dc in range(4):
                        nc.tensor.matmul(
                            ps[:],
                            kt[:, dc, kj * 128 : (kj + 1) * 128],
                            qt[:, dc, qsl],
                            start=(dc == 0),
                            stop=(dc == 3),
                        )
                    pt = ptp.tile([128, 512], F16, tag="pt")
                    nc.scalar.activation(pt[:], ps[:], AF.Exp, scale=SCALE)
                    nc.vector.tensor_mul(pt[:], pt[:], mask_sb[:, kj, qsl])
                    if kj == 0:
                        nc.vector.tensor_copy(acc[:], pt[:])
                    else:
                        nc.vector.tensor_add(acc[:], acc[:], pt[:])
                    pts.append(pt)
                rs = psrow.tile([1, 512], F32, tag="row")
                nc.tensor.matmul(
                    rs[:], ones[:, 0:1], acc[:], start=True, stop=True
                )
                rec = small.tile([1, 512], F32, tag="rec")
                nc.vector.reciprocal_approx_fast(rec[:], rs[:])
                rb = small.tile([128, 512], F32, tag="rb")
                nc.gpsimd.partition_broadcast(rb[:], rec[:])
                for dj in range(4):
                    cps = psum.tile([128, 512], F32, tag="mm")
                    for kj in range(8):
                        nc.tensor.matmul(
                            cps[:],
                            vt[:, kj, dj * 128 : (dj + 1) * 128],
                            pts[kj][:],
                            start=(kj == 0),
                            stop=(kj == 7),
                        )
                    nc.vector.tensor_mul(ctxn[:, dj, qsl], cps[:], rb[:])
            state[h]["ctxn"] = ctxn

        def outproj(h):
            wp = wpool.tile([128, 4, D], F16, tag="wp")
            for dj in range(4):
                nc.sync.dma_start(wp[:, dj, :], wp_d[h, dj])
            ctxn = state[h]["ctxn"]
            for qm in range(8):
                ps = psum.tile([128, 512], F32, tag="mm")
                for dj in range(4):
                    nc.tensor.matmul(
                        ps[:],
                        ctxn[:, dj, qm * 128 : (qm + 1) * 128],
                        wp[:, dj, :],
                        start=(dj == 0),
                        stop=(dj == 3 and h != 0),
                    )
                if h == 0:
                    nc.tensor.matmul(
                        ps[:], ones[0:1, 0:128], bp_sb[:], start=False, stop=True
                    )
                    nc.scalar.copy(out_acc[:, qm, :], ps[:])
                else:
                    nc.vector.tensor_add(
                        out_acc[:, qm, :], out_acc[:, qm, :], ps[:]
                    )
                if h == H - 1:
                    nc.sync.dma_start(
                        out_d[qm * 128 : (qm + 1) * 128, :], out_acc[:, qm, :]
                    )
            del state[h]["qt"], state[h]["kt"], state[h]["vt"], state[h]["ctxn"]

        for h in range(H):
            proj(h)
            if h == 0:
                load_consts()
            if h > 0:
                outproj(h - 1)
            attn(h)
        outproj(H - 1)

    nc.compile()
    return nc


def _get_program(use_bias):
    key = ("nc", use_bias)
    if key not in _cache:
        _cache[key] = _build_bias() if use_bias else _build_fast()
    return _cache[key]


def kernel(x, states, mask, Wq, bq, Wk, bk, Wv, bv, Wp, bp):
    global LAST_EXEC_NS

    x = np.asarray(x, dtype=np.float32)
    states = np.asarray(states, dtype=np.float32)
    mask = np.asarray(mask)
    f32 = np.float32
    bq_np, bk_np = np.asarray(bq, f32), np.asarray(bk, f32)
    bv_np, bp_np = np.asarray(bv, f32), np.asarray(bp, f32)
    use_bias = bool(
        bq_np.any() or bk_np.any() or bv_np.any() or bp_np.any()
    )
    nc = _get_program(use_bias)

    if use_bias:
        wq_np = np.ascontiguousarray(
            np.asarray(Wq, f32).transpose(0, 2, 1)
        ).reshape(H, 4, 128, D).astype(np.float16)
        wk_np = np.ascontiguousarray(
            np.asarray(Wk, f32).transpose(0, 2, 1)
        ).reshape(H, 4, 128, D).astype(np.float16)
        wv_np = np.ascontiguousarray(
            np.asarray(Wv, f32).transpose(0, 2, 1)
        ).reshape(H, 4, 128, D).astype(np.float16)
        wp_np = np.ascontiguousarray(np.asarray(Wp, f32).T).reshape(
            H, 4, 128, D
        ).astype(np.float16)
        shared = {
            "wqT": wq_np,
            "wkT": wk_np,
            "wvT": wv_np,
            "wpT": wp_np,
            "ones": np.ones((128, 512), f32),
            "bq": bq_np,
            "bk": bk_np,
            "bv": bv_np,
            "bp": bp_np.reshape(1, D),
        }
        in_maps = []
        for b in range(B):
            xT = np.ascontiguousarray(x[b].T).reshape(4, 128, L).astype(np.float16)
            sT = np.ascontiguousarray(states[b].T).reshape(4, 128, L).astype(
                np.float16
            )
            mT = np.ascontiguousarray(mask[b].T).astype(np.float16).reshape(
                8, 128, L
            )
            in_maps.append({"xT": xT, "sT": sT, "maskT": mT, **shared})
    else:
        # Natural [d, e] layouts for the folded-weight chains.
        wq_np = np.asarray(Wq, f32).reshape(H, 4, 128, E).astype(np.float16)
        wk_np = np.asarray(Wk, f32).reshape(H, 4, 128, E).astype(np.float16)
        wv_np = np.asarray(Wv, f32).reshape(H, 4, 128, E).astype(np.float16)
        wp_np = np.ascontiguousarray(np.asarray(Wp, f32).T).reshape(
            H, 4, 128, D
        ).astype(np.float16)
        shared = {
            "wq": wq_np,
            "wk": wk_np,
            "wv": wv_np,
            "wpT": wp_np,
            "ones": np.ones((128, 512), f32),
        }
        in_maps = []
        for b in range(B):
            xT = np.ascontiguousarray(x[b].T).reshape(4, 128, L).astype(np.float16)
            sT = np.ascontiguousarray(states[b].T).reshape(4, 128, L).astype(
                np.float16
            )
            sNb = np.ascontiguousarray(states[b]).reshape(8, 128, E).astype(
                np.float16
            )
            mT = np.ascontiguousarray(mask[b].T).astype(np.float16).reshape(
                8, 128, L
            )
            in_maps.append(
                {"xT": xT, "sT": sT, "sN": sNb, "maskT": mT, **shared}
            )

    res = run_bass_kernel_spmd(
        nc, in_maps, core_ids=list(range(NCORES)), trace=TRACE
    )
    LAST_EXEC_NS = res.exec_time_ns
    return np.stack([res.results[b]["out"] for b in range(B)], axis=0)
